# revision 103
# baseline (speedup 1.0000x reference)
"""Trainium2 Bass kernel for nn_Attention_5463198400554.

Reference computation (per batch b of 8):
    q    = Wq @ x[b]                      # (N, C) contraction over x's first axis
    attn = scale * q @ x[b].T             # (N, N) contraction over x's second axis
    m    = rowmax(attn)                   # (N, 1)
    v    = colmean(x[b])                  # (1, C)  (mean over tokens)
    out[b][i][j] = v[i] * m[j]            # outer product, (C, N) == (N, C)

Strategy: pure data-parallel over batch — 8 batches on 8 NeuronCores, no
collectives. Key algebraic move: attn = scale * Wq @ G with G = x @ x.T
symmetric, so q is never computed; only G's upper block-triangle is built
by matmul and the strictly-lower 128-blocks are mirrored by PE transposes
(regular bf16 matmuls against an identity). Both x and Wq are passed
pre-transposed by the host (pure layout marshalling in kernel()), so the
kernel does no input transposes at all.

Pipeline (single TileContext; Tile owns all semaphores):
  1. xT streams in as one strided [2048, 128] DMA per 128-token block
     (all channels of the block at once), cast f32->bf16 into xt by a
     single Pool/ACT op pair per block. Token blocks 0..7 run a
     block-pair G ramp: as each single block lands, all block-pair G
     units against earlier blocks are emitted (into a staging square,
     since g's SBUF only frees once x staging closes), so the PE has
     near-quadratic work growth during the DMA-bound load instead of
     waiting for full 512-token groups.
  2. Remaining G upper chunks ([128,512] psum accum over c-blocks) with
     lower-mirrors flushed as dependencies complete; WqT half-row pieces
     DMA on the idle SP queue and cast on Pool/ACT; v (column sums of x)
     runs on ACT via activation accum_out halves (keeping DVE free for
     the evacuations that gate mirrors), combined once on DVE.
  3. attn in (nb, mc) [128,512] chunks, one PSUM bank each, mc-outer with
     per-chunk partial rowmax (DVE) into m4_all. Hybrid precision: per
     chunk the contraction half whose rows contain mc's diagonal runs in
     bf16 (6 row-blocks); the other 10 row-blocks run as 5 fp8(e4m3)
     DoubleRow matmuls (2 k-tiles per pass, ~2x PE throughput), with
     scales g8 = G/16, wq8 = Wq*16 cancelling exactly. The G diagonal
     (~2048 vs off-diag sigma ~45) never sees fp8; measured HW rel err
     1.47e-2 vs the 2e-2 budget. The fp8 copies are cast from bf16 right
     after xt's SBUF frees; the first 4 chunks run fully in bf16 so the
     PE has work while casts run. Two attn PSUM banks are pre-allocated
     for the whole kernel so early chunks co-schedule into the late G
     phase without PSUM WAR stalls. Mirror flushing is delayed until 8
     key-2 chunks are in the PE stream: engine streams execute strictly
     in order, and the first mirrors read g regions WAR-gated on the
     x-staging pool boundary — emitted early they would head-of-line
     block the already-ready chunk matmuls behind them.
  4. Per-nb epilogue: combine partial maxes, broadcast m across
     partitions (DVE 32x32 stream-transpose + DRAM bounce for most
     blocks; a low-latency PE transpose + K=1 ones-matmul broadcast for
     the last two), then quarter-granular fused scalar_tensor_tensor
     stores -> 256KB output DMAs, keeping the kernel tail short.

The walrus build here caps sync waits at 1 per instruction (2 for
EventSemaphore); _legalize_wait_counts splits Tile's over-capacity waits
onto injected same-engine EventSemaphore carriers post-scheduling.
"""

from contextlib import ExitStack

import numpy as np

import concourse.bass as bass
import concourse.tile as tile
from concourse import mybir
from concourse.bass_utils import run_bass_kernel_spmd
from concourse.masks import make_identity


def _legalize_wait_counts(nc: bass.Bass) -> None:
    """Split over-capacity sync waits onto injected EventSemaphore carriers.

    This walrus build rejects instructions carrying more sync waits than the
    ISA struct holds ("Too many sync wait commands"): 1 wait for ordinary
    instructions, 2 for EventSemaphore. Tile's wait assignment emits more
    (e.g. WAR + RAW on one DMA, or the kernel-tail Drain waiting on every
    DMA queue). Moving excess waits to same-engine EventSemaphore carriers
    immediately before the instruction preserves ordering: the engine blocks
    until those semaphores reach their thresholds, then issues the original
    instruction with the remaining wait.
    """
    counter = [0]
    for blk in nc.m.functions[0].blocks:
        new_insts = []
        changed = False
        for ins in blk.instructions:
            si = ins.sync_info
            waits = list(si.on_wait) if si is not None else []
            cap = 2 if isinstance(ins, mybir.InstEventSemaphore) else 1
            if len(waits) > cap:
                changed = True
                excess, keep = waits[:-cap], waits[-cap:]
                for s in range(0, len(excess), 2):
                    counter[0] += 1
                    ev = mybir.InstEventSemaphore(
                        name=f"waitsplit-{counter[0]}", ins=[], outs=[]
                    )
                    ev.engine = ins.engine
                    ev.sync_info = mybir.SyncInfo(
                        on_wait=excess[s : s + 2], on_update=[]
                    )
                    new_insts.append(ev)
                ins.sync_info = mybir.SyncInfo(
                    on_wait=keep, on_update=list(si.on_update)
                )
            new_insts.append(ins)
        if changed:
            blk.instructions = new_insts

MARKS = []  # (tag, next-inst-id) snapshots for offline cost attribution


def _mark(nc, tag):
    MARKS.append((tag, nc.next_id()))


B = 8
N = 2048  # tokens == channels == dim
P = 128  # partitions
NB = N // P  # 16 blocks of 128
OC = 512  # matmul moving-operand chunk (one PSUM bank of f32)
NOC = N // OC  # 4 chunks
NUM_HEADS = 8
SCALE = (N // NUM_HEADS) ** -0.5  # 1/16
OUT_CONST = SCALE / N  # folds attn scale and the v-mean divisor

F32 = mybir.dt.float32
BF16 = mybir.dt.bfloat16
FP8 = mybir.dt.float8e4


def build_graph(reps: int = 1) -> bass.Bass:
    nc = bass.Bass(trn_type="TRN2", target_bir_lowering=False, debug=False)
    # Both operands arrive pre-transposed from the host (pure layout
    # marshalling): xt_ext[c, m] = x[m, c] and wqt_ext[i, o] = Wq[o, i], so
    # contraction rows land on partitions directly and no on-device
    # transposes are needed at all.
    xt_ext = nc.dram_tensor("xt", [N, N], F32, kind="ExternalInput").ap()
    wqt_ext = nc.dram_tensor("wqt", [N, N], F32, kind="ExternalInput").ap()
    out_ext = nc.dram_tensor("out", [N, N], F32, kind="ExternalOutput").ap()

    with tile.TileContext(nc) as tc, ExitStack() as octx:
        consts = octx.enter_context(tc.tile_pool(name="consts", bufs=1))
        ident_bf = consts.tile([P, P], BF16, name="ident_bf")
        make_identity(nc, ident_bf)
        ident_f32 = consts.tile([P, P], F32, name="ident_f32")
        make_identity(nc, ident_f32)
        ones_f32 = consts.tile([1, P], F32, name="ones_f32")
        nc.vector.memset(ones_f32[:], 1.0)
        for rep in range(reps):
            _emit_body(
                nc, tc, xt_ext, wqt_ext, out_ext, ident_bf, ident_f32, ones_f32, rep
            )

    _legalize_wait_counts(nc)
    return nc


def _emit_body(nc, tc, xt_ext, wqt_ext, out_ext, ident_bf, ident_f32, ones_f32, rep):
    """attn = scale * Wq @ G with G = x @ x.T (symmetric); see module doc."""
    R = f"r{rep}_"
    with ExitStack() as ctx:
        stats = ctx.enter_context(tc.tile_pool(name=R + "stats", bufs=1))
        dram = ctx.enter_context(tc.tile_pool(name=R + "dram", bufs=16, space="DRAM"))

        v_all = stats.tile([P, NB], F32, name=R + "v_all")  # column sums of x
        v_parts = stats.tile([P, NB, 2], F32, name=R + "v_parts")
        # per-(nb, mc) partial row maxes of attn
        m4_all = stats.tile([P, NB, NOC], F32, name=R + "m4_all")

        wqt_pool = ctx.enter_context(
            tc.tile_pool(name=R + "wqt", bufs=1, side="right")
        )
        wqt = wqt_pool.tile([P, NB, N], BF16, name=R + "wqt")  # WqT[i, n]
        g = None

        # attn chunk PSUM: allocated up front (own 2 banks) so interleaved
        # attn chunks never wait on G-phase PSUM WAR chains
        psb1_pool = ctx.enter_context(
            tc.tile_pool(name=R + "psB1", bufs=2, space="PSUM")
        )

        with tc.tile_pool(name=R + "xt", bufs=1) as xt_pool:
            xt = xt_pool.tile([P, NB, N], BF16, name=R + "xt")  # xT[c, m]

            # ---- load x, cast, transpose into xt ----
            # psX (transposes) and psG (G accumulation) coexist so G chunks
            # can start filling PE gaps while later x-groups still stream in.
            pctx = ExitStack()
            psg_pool = pctx.enter_context(
                tc.tile_pool(name=R + "psG", bufs=6, space="PSUM")
            )
            g0ctx = ExitStack()
            g0_pool = g0ctx.enter_context(tc.tile_pool(name=R + "g0", bufs=1))
            # staging for the early-ramp G blocks (g proper is not yet
            # allocated during the load phase): rows 0..3 x cols 0..3
            # (group-0 square incl. mirrors) and rows 0..7 x cols 4..7
            # (group-1 uppers; mirrors deferred to flush_low). Copied into
            # g after the x staging pools close.
            # gsq rows 0..3: group-0 square (G cols 0..3); rows 4..11:
            # group-1 uppers, i.e. G rows 0..7 x cols 4..7 at gsq row 4+a
            gsq = g0_pool.tile([P, 12, OC], BF16, name=R + "gsq")
            with tc.tile_pool(name=R + "xs", bufs=5) as xs_pool:
                # x arrives pre-transposed: one strided [2048, 128] DMA per
                # token block lands ALL channels of that block at once, and a
                # single cast (split Pool/ACT by halves) writes it into xt.
                # No PE transposes, no per-s evacuations.
                for i in range(NB):  # token blocks
                    xs = xs_pool.tile([P, NB, P], F32, tag="xs", name=f"{R}xs{i}")
                    _mark(nc, "x_dma")
                    nc.sync.dma_start(
                        xs[:],
                        xt_ext[:, i * P : (i + 1) * P].rearrange(
                            "(s p) t -> p s t", p=P
                        ),
                    )
                    _mark(nc, "x_cast")
                    nc.gpsimd.tensor_copy(
                        xt[:, 0 : NB // 2, i * P : (i + 1) * P],
                        xs[:, 0 : NB // 2, :],
                    )
                    _mark(nc, "x_cast")
                    nc.scalar.copy(
                        xt[:, NB // 2 : NB, i * P : (i + 1) * P],
                        xs[:, NB // 2 : NB, :],
                    )
                    if i >= 8:
                        continue
                    # token blocks 0..7 are the pipeline ramp: block-pair G
                    # units start as soon as each single block lands, instead
                    # of waiting for a whole 512-token group.
                    ig = i // 4
                    for a in range(i + 1):
                        pgp = psg_pool.tile(
                            [P, P], F32, tag="pg", name=f"{R}pgp{a}_{i}"
                        )
                        _mark(nc, "g_mm")
                        for cb in range(NB):
                            nc.tensor.matmul(
                                pgp[:],
                                xt[:, cb, a * P : (a + 1) * P],
                                xt[:, cb, i * P : (i + 1) * P],
                                start=(cb == 0),
                                stop=(cb == NB - 1),
                            )
                        # gsq row: group 0 -> G row a (cols 0..3); group 1
                        # -> 4 + a (cols 4..7)
                        gr = a if ig == 0 else 4 + a
                        gc = (i - 4 * ig) * P
                        _mark(nc, "g_evac")
                        nc.vector.tensor_copy(gsq[:, gr, gc : gc + P], pgp[:])
                        if ig == 0 and a < i:
                            # in-square mirror (group 0 only; group-1
                            # mirrors defer to flush_low once g is up)
                            plp = psg_pool.tile(
                                [P, P], F32, tag="pg", name=f"{R}plp{i}_{a}"
                            )
                            _mark(nc, "low_mm")
                            nc.tensor.matmul(
                                plp[:],
                                gsq[:, a, gc : gc + P],
                                ident_bf[:],
                                start=True,
                                stop=True,
                            )
                            _mark(nc, "low_evac")
                            nc.vector.tensor_copy(
                                gsq[:, i, a * P : (a + 1) * P], plp[:]
                            )

            # ---- G = x @ x.T upper chunks; Wq stage emitted after so the
            #      PE prefers G matmuls while Wq DMA streams ----
            g_pool = ctx.enter_context(
                tc.tile_pool(name=R + "g", bufs=1, side="right")
            )
            g = g_pool.tile([P, NB, N], BF16, name=R + "g")  # G[n, m]
            _copies = [(a, 0, a) for a in range(4)]  # (G row, col off, gsq row)
            _copies += [(a, OC, 4 + a) for a in range(8)]
            for ci, (a, co, gr) in enumerate(_copies):
                _mark(nc, "g0_copy")
                if ci % 3 == 0:
                    nc.gpsimd.tensor_copy(g[:, a, co : co + OC], gsq[:, gr, :])
                elif ci % 3 == 1:
                    nc.vector.tensor_copy(g[:, a, co : co + OC], gsq[:, gr, :])
                else:
                    nc.scalar.copy(g[:, a, co : co + OC], gsq[:, gr, :])
            g0ctx.close()
            with (
                tc.tile_pool(name=R + "wqs", bufs=2) as wqs_pool,
                tc.tile_pool(name=R + "vscr", bufs=2) as vscr_pool,
            ):

                def emit_g_chunk(a, bc):
                    # diagonal chunk starts at the diagonal block; the skipped
                    # sub-diagonal blocks are mirrored from column a instead
                    off = (a % 4) * P if bc == a // 4 else 0
                    pg = psg_pool.tile([P, OC], F32, tag="pg", name=f"{R}pg{a}_{bc}")
                    _mark(nc, "g_mm")
                    for cb in range(NB):
                        nc.tensor.matmul(
                            pg[:, off:OC],
                            xt[:, cb, a * P : (a + 1) * P],
                            xt[:, cb, bc * OC + off : (bc + 1) * OC],
                            start=(cb == 0),
                            stop=(cb == NB - 1),
                        )
                    _mark(nc, "g_evac")
                    nc.vector.tensor_copy(
                        g[:, a, bc * OC + off : (bc + 1) * OC], pg[:, off:OC]
                    )

                WH = N // 2

                def emit_wq_piece(s, h, unit):
                    # WqT arrives pre-transposed: DMA a half row-block on the
                    # idle SP queue, cast f32->bf16 on Pool/ACT
                    ws = wqs_pool.tile([P, WH], F32, tag="ws", name=f"{R}ws{s}_{h}")
                    _mark(nc, "wq_dma")
                    nc.sync.dma_start(
                        ws[:], wqt_ext[s * P : (s + 1) * P, h * WH : (h + 1) * WH]
                    )
                    _mark(nc, "wq_cast")
                    if unit % 3 == 2:
                        nc.scalar.copy(wqt[:, s, h * WH : (h + 1) * WH], ws[:])
                    else:
                        nc.gpsimd.tensor_copy(
                            wqt[:, s, h * WH : (h + 1) * WH], ws[:]
                        )

                def emit_g_low(a, bg, w):
                    pl = psg_pool.tile(
                        [P, OC], F32, tag="pg", name=f"{R}pl{a}_{bg}"
                    )
                    _mark(nc, "low_mm")
                    for k in range(w):
                        b = bg * 4 + k
                        nc.tensor.matmul(
                            pl[:, k * P : (k + 1) * P],
                            g[:, b, a * P : (a + 1) * P],
                            ident_bf[:],
                            start=True,
                            stop=True,
                        )
                    _mark(nc, "low_evac")
                    nc.vector.tensor_copy(
                        g[:, a, bg * OC : bg * OC + w * P], pl[:, 0 : w * P]
                    )

                # ordered so chunk (a, bc) is emitted once x-groups
                # max(a//4, bc) have landed -> G starts after group 0.
                # The (a<4, bc=0) square and the (a<8, bc=1) uppers were
                # already built block-pair-wise during the load ramp (gsq).
                g_chunks = sorted(
                    (
                        (a, bc)
                        for a in range(NB)
                        for bc in range(a // 4, NOC)
                        if not (a < 4 and bc == 0) and not (a < 8 and bc == 1)
                    ),
                    key=lambda t: (max(t[0] // 4, t[1]), t[1], t[0]),
                )
                # lower-mirror group (a, bg, w) covers blocks b in
                # [4bg, 4bg+w); depends on upper chunks (b, a//4).
                # a<4 partial mirrors were handled in the gsq ramp square.
                low_pending = [
                    (a, bg, 4) for a in range(NB) for bg in range(a // 4)
                ]
                low_pending += [
                    (a, a // 4, a % 4) for a in range(4, NB) if a % 4 > 0
                ]
                done_chunks = {(a, 0) for a in range(4)}
                done_chunks |= {(a, 1) for a in range(8)}

                def flush_low():
                    nonlocal low_pending
                    rest = []
                    for a, bg, w in low_pending:
                        deps = {(4 * bg + k, a // 4) for k in range(w)}
                        if deps <= done_chunks:
                            emit_g_low(a, bg, w)
                        else:
                            rest.append((a, bg, w))
                    low_pending = rest

                # v: column sums of x == row sums of xT. Runs on ACT (idle
                # mid-G) as two half-row activation+accumulate passes per s,
                # keeping the DVE stream free for the evacuations that gate
                # mirror matmuls; partials combine once on DVE at the end.
                VH = N // 2

                def emit_v(s):
                    for hh in range(2):
                        vs = vscr_pool.tile(
                            [P, VH], BF16, tag="vs", name=f"{R}vs{s}_{hh}"
                        )
                        _mark(nc, "v")
                        nc.scalar.activation(
                            out=vs[:],
                            in_=xt[:, s, hh * VH : (hh + 1) * VH],
                            func=mybir.ActivationFunctionType.Copy,
                            accum_out=v_parts[:, s, hh : hh + 1],
                        )

                # wq pieces carry no PE work (pre-transposed): G chunks and
                # mirrors drive the stream; pieces and v ops sprinkle in.
                wq_pieces = [(s, h) for s in range(NB) for h in range(2)]
                pi = 0
                v_next = 0
                for gi in range(len(g_chunks)):
                    emit_g_chunk(*g_chunks[gi])
                    done_chunks.add(g_chunks[gi])
                    # the first mirror flush reads g regions still WAR-gated
                    # on the x-staging pool boundary; emitting those PE
                    # transposes early would head-of-line-block the (ready)
                    # key-2 chunk matmuls behind them in the PE stream
                    if gi >= 7:
                        flush_low()
                    for _ in range(2):
                        if pi < len(wq_pieces):
                            emit_wq_piece(*wq_pieces[pi], pi)
                            pi += 1
                    if gi >= 12 and v_next < NB:
                        emit_v(v_next)
                        v_next += 1
                assert not low_pending
                while pi < len(wq_pieces):
                    emit_wq_piece(*wq_pieces[pi], pi)
                    pi += 1
                while v_next < NB:
                    emit_v(v_next)
                    v_next += 1
                _mark(nc, "v")
                nc.vector.reduce_sum(
                    out=v_all[:], in_=v_parts[:], axis=mybir.AxisListType.X
                )

        pctx.close()

        # ---- attn chunks, rowmax combine, column-wise epilogue ----
        # mc-outer: each (nb, mc) 512-col chunk accumulates into a single
        # PSUM bank with a partial rowmax per chunk; short kernel tail.
        # Hybrid precision: per chunk, the contraction half whose rows
        # contain the chunk's diagonal runs in bf16; the other half runs as
        # fp8(e4m3) DoubleRow matmuls (2 k-tiles per pass). The diagonal
        # (large, 2048 vs sigma 45) therefore never sees fp8. Scales
        # g8 = G/16, wq8 = Wq*16 cancel exactly.
        with (
            tc.tile_pool(name=R + "psB", bufs=4, space="PSUM") as psb_pool,
            tc.tile_pool(name=R + "psE", bufs=1, space="PSUM") as pse_pool,
            tc.tile_pool(name=R + "epi", bufs=3) as epi_pool,
            tc.tile_pool(name=R + "ot", bufs=4) as ot_pool,
            tc.tile_pool(name=R + "f8", bufs=1) as f8_pool,
        ):
            g8 = f8_pool.tile([P, NB, N], FP8, name=R + "g8")
            wqt8 = f8_pool.tile([P, NB, N], FP8, name=R + "wqt8")

            # per column-chunk mc: 6 bf16 rows (covering mc's diagonal
            # blocks 4mc..4mc+3) and 10 fp8 rows as 5 DoubleRow pairs
            BF_ROWS = {0: range(0, 6), 1: range(2, 8), 2: range(8, 14), 3: range(10, 16)}
            FP8_PAIRS = {
                0: (6, 8, 10, 12, 14),
                1: (0, 8, 10, 12, 14),
                2: (0, 2, 4, 6, 14),
                3: (0, 2, 4, 6, 8),
            }

            # casts, ordered by first use: rows 8..15 (mc 0/1), then 6..7,
            # 0..1, 2..5
            for i, s in enumerate([8, 9, 10, 11, 12, 13, 14, 15, 6, 7, 0, 1, 2, 3, 4, 5]):
                _mark(nc, "f8cast")
                if i % 3 == 0:
                    nc.vector.tensor_scalar_mul(g8[:, s, :], g[:, s, :], 1 / 16.0)
                    nc.vector.tensor_scalar_mul(wqt8[:, s, :], wqt[:, s, :], 16.0)
                elif i % 3 == 1:
                    nc.scalar.mul(g8[:, s, :], g[:, s, :], 1 / 16.0)
                    nc.scalar.mul(wqt8[:, s, :], wqt[:, s, :], 16.0)
                else:
                    nc.gpsimd.tensor_scalar_mul(g8[:, s, :], g[:, s, :], 1 / 16.0)
                    nc.gpsimd.tensor_scalar_mul(wqt8[:, s, :], wqt[:, s, :], 16.0)

            def emit_attn_chunk(nb, mc, pool, full_bf16=False):
                pb = pool.tile([P, OC], F32, tag="pb", name=f"{R}pb{nb}_{mc}")
                bf_rows = range(NB) if full_bf16 else BF_ROWS[mc]
                _mark(nc, "attn_mm")
                for k, ib in enumerate(bf_rows):
                    nc.tensor.matmul(
                        pb[:],
                        wqt[:, ib, nb * P : (nb + 1) * P],
                        g[:, ib, mc * OC : (mc + 1) * OC],
                        start=(k == 0),
                        stop=(full_bf16 and k == NB - 1),
                    )
                if not full_bf16:
                    pairs = FP8_PAIRS[mc]
                    for j, s0 in enumerate(pairs):  # fp8 DoubleRow k-tile pairs
                        nc.tensor.matmul(
                            pb[:],
                            wqt8[:, s0 : s0 + 2, nb * P : (nb + 1) * P],
                            g8[:, s0 : s0 + 2, mc * OC : (mc + 1) * OC],
                            start=False,
                            stop=(j == len(pairs) - 1),
                            perf_mode=mybir.MatmulPerfMode.DoubleRow,
                        )
                _mark(nc, "rowmax")
                nc.vector.reduce_max(
                    out=m4_all[:, nb, mc : mc + 1],
                    in_=pb[:],
                    axis=mybir.AxisListType.X,
                )

            def emit_epilogue_bounce(nb, mt_in):
                # m column -> row strips via DVE 32x32 stream transpose:
                # mt[32b, c] = m[32b + c]; 4-descriptor DMA to a DRAM row,
                # then partition-broadcast load back.
                mt = epi_pool.tile([P, 32], F32, tag="mt", name=f"{R}mt{nb}")
                _mark(nc, "epi")
                nc.vector.transpose(mt[:], mt_in[:])
                md = dram.tile([1, P], F32, tag="md", name=f"{R}md{nb}")
                strips = bass.AP(
                    tensor=mt.tensor,
                    offset=mt.offset,
                    ap=[[32 * mt.ap[0][0], 4], [1, 32]],
                )
                nc.sync.dma_start(md[0, :].rearrange("(a b) -> a b", a=4), strips)
                m_bc = epi_pool.tile([P, P], F32, tag="mbc", name=f"{R}mb{nb}")
                nc.sync.dma_start(
                    m_bc[:],
                    bass.AP(tensor=md.tensor, offset=md.offset, ap=[[0, P], [1, P]]),
                )
                return m_bc

            def emit_epilogue_pe(nb, mt_in):
                # PE path (short latency, used for the tail blocks): transpose
                # the m column to a PSUM row, bounce through SBUF, then a K=1
                # ones-matmul replicates it across all 128 partitions.
                _mark(nc, "epi")
                pmr = pse_pool.tile([1, P], F32, tag="pmr", name=f"{R}pmr{nb}")
                nc.tensor.matmul(
                    pmr[:], mt_in[:, 0:1], ident_f32[:], start=True, stop=True
                )
                smr = epi_pool.tile([1, P], F32, tag="smr", name=f"{R}smr{nb}")
                nc.vector.tensor_copy(smr[:], pmr[:])
                m_bc = pse_pool.tile([P, P], F32, tag="pbc", name=f"{R}pbc{nb}")
                nc.tensor.matmul(
                    m_bc[:], ones_f32[:], smr[:], start=True, stop=True
                )
                return m_bc

            def emit_store(nb, m_bc, ib0, ib1, queue=None):
                # out rows [ib0*128, ib1*128) of column block nb; quarter-
                # granular ot tiles keep SBUF small and the tail short
                w = ib1 - ib0
                ot = ot_pool.tile([P, 4, P], F32, tag="ot", name=f"{R}ot{nb}_{ib0}")
                m_in = bass.AP(
                    tensor=m_bc.tensor,
                    offset=m_bc.offset,
                    ap=[m_bc.ap[0], [0, w], [1, P]],
                )
                v_in = bass.AP(
                    tensor=v_all.tensor,
                    offset=v_all.offset + ib0 * v_all.ap[1][0],
                    ap=[v_all.ap[0], [v_all.ap[1][0], w], [0, P]],
                )
                _mark(nc, "stt")
                nc.vector.scalar_tensor_tensor(
                    out=ot[:, 0:w, :],
                    in0=m_in,
                    scalar=OUT_CONST,
                    in1=v_in,
                    op0=mybir.AluOpType.mult,
                    op1=mybir.AluOpType.mult,
                )
                _mark(nc, "out_dma")
                (queue or nc.sync).dma_start(
                    out_ext[
                        ib0 * P : ib1 * P, nb * P : (nb + 1) * P
                    ].rearrange("(ib p) j -> p ib j", p=P),
                    ot[:, 0:w, :],
                )

            nchunk = 0
            for nb in range(NB):
                for mc in range(NOC):
                    # rotate chunks over the two PSUM pools (2 + 4 banks);
                    # the first chunks run fully in bf16 so the PE has work
                    # while the fp8 casts (gated on xt's SBUF freeing) run
                    pool = psb1_pool if nchunk % 3 == 2 else psb_pool
                    emit_attn_chunk(nb, mc, pool, full_bf16=nchunk < 4)
                    nchunk += 1
                mt_in = epi_pool.tile([P, 32], F32, tag="mti", name=f"{R}mti{nb}")
                _mark(nc, "rowmax")
                nc.vector.reduce_max(
                    out=mt_in[:, 0:1],
                    in_=m4_all[:, nb, 0:NOC],
                    axis=mybir.AxisListType.X,
                )
                if nb < NB - 2:
                    m_bc = emit_epilogue_bounce(nb, mt_in)
                    for qs in range(4):
                        emit_store(nb, m_bc, qs * 4, (qs + 1) * 4)
                else:
                    # tail blocks: low-latency PE broadcast path
                    m_bc = emit_epilogue_pe(nb, mt_in)
                    for qs in range(4):
                        emit_store(nb, m_bc, qs * 4, (qs + 1) * 4)
                _mark(nc, "other")


_NC_CACHE = None


def _get_graph() -> bass.Bass:
    global _NC_CACHE
    if _NC_CACHE is None:
        _NC_CACHE = build_graph()
    return _NC_CACHE


def kernel(x=None, Wq=None, H=None, W=None, **_ignored) -> np.ndarray:
    """Full-input entry point: x (8, 2048, 2048) f32, Wq (2048, 2048) f32.

    Shards batch elements across the 8 NeuronCores (data parallel), runs the
    Bass kernel SPMD, and stacks the per-core outputs back to (8, 2048, 2048).
    H and W are unused by the computation (the reference ignores them).
    """
    x = np.asarray(x, dtype=np.float32)
    wq = np.asarray(Wq, dtype=np.float32)
    assert x.shape == (B, N, N) and wq.shape == (N, N)
    # layout marshalling for the device: both operands pre-transposed
    xts = np.ascontiguousarray(np.swapaxes(x, 1, 2))
    wqt = np.ascontiguousarray(wq.T)

    nc = _get_graph()
    in_maps = [{"xt": xts[c], "wqt": wqt} for c in range(B)]
    res = run_bass_kernel_spmd(nc, in_maps, core_ids=list(range(B)))
    return np.stack([res.results[c]["out"] for c in range(B)], axis=0)


if __name__ == "__main__":
    rng = np.random.default_rng(0)
    x = rng.standard_normal((B, N, N), dtype=np.float32)
    wq = (rng.standard_normal((N, N), dtype=np.float32) * 0.02).astype(np.float32)
    out = kernel(x=x, Wq=wq, H=64, W=32)
    print("out shape:", out.shape, out.dtype)

